# revision 1
# baseline (speedup 1.0000x reference)
"""BitLinear kernel for Trainium2, 8-core column-parallel.

Computes out = x @ (sign(W) * (weight_scale @ input_factor)).T
  x: [32, 8, 4096] f32, W: [11008, 4096] f32,
  weight_scale: [11008, 4] f32, input_factor: [4, 4096] f32
  -> out: [32, 8, 11008] f32

Sharding: column-parallel over out_features (11008 = 8 x 1376). Each core
gets its W / weight_scale row-shard plus replicated x / input_factor, and
produces out[:, core_slice]; host concatenates. No collectives.

Per-core dataflow (all on-device):
  - W ships as bf16 (sign-exact cast; only sign(W) is consumed) and is
    transposed by the DMA xbar on load: [128 i, 1376 o] strips, one per
    i-block. Halves the HBM stream and needs no PE transposes.
  - PE computes value strips value[i_blk, o_chunk] = f.T @ wsT (K=4 matmul)
  - ACT extracts s = sign(w) in {-1, 0, +1} via the Sign LUT (sign(+-0)=0,
    matching jnp.sign)
  - DVE multiplies w_signed = s * value (output cast to the matmul dtype)
  - PE main matmuls run as one dense burst per i-block:
    out[t, o] += xT_blk.T @ w_signed, accumulated in PSUM over all 32
    i-blocks (2 token-blocks x 3 banks + 2 value banks = 8 PSUM banks),
    then evacuated + DMA'd out.
Matmul operands use fp16 (11 mantissa bits, ~5e-4 rel err, full PE rate);
set BITLINEAR_PRECISION=f32 for exact-but-4x-slower fp32 matmuls.
"""

import os
import sys

if "/opt/trn_rl_repo" not in sys.path:
    sys.path.insert(0, "/opt/trn_rl_repo")

import numpy as np

# ---------------------------------------------------------------------------
# problem constants (hardcoded per the self-contained-kernel contract)
B, S, IN, OUT, R = 32, 8, 4096, 11008, 4
T = B * S               # 256 tokens
NCORES = 8
OS = OUT // NCORES      # 1376 out-features per core
P = 128
IC = 2048               # i-span per W DMA macro-tile
O_CHUNKS = [(0, 512), (512, 512), (1024, 352)]

# matmul precision mode:
#   "f32"  - plain fp32 matmuls (exact, ~1e-6 rel err) but TensorE runs
#            fp32 at 4 cycles/row -> PE-bound ~340us.
#   "f32r" - TF32-like fp32r (11 mantissa bits, 1 cycle/row at N>=256),
#            ~5e-4 rel err, ~3x faster. Well inside the 2e-2 gate.
PRECISION = os.environ.get("BITLINEAR_PRECISION", "f16")


def _install_tile_drain_patch():
    """This walrus build rejects >2 sync waits on one TPB_CTRL instruction;
    split the TileContext end-of-kernel drain into one drain per proc."""
    from concourse.tile import TileContext
    from concourse.vector_clock import ScopedClock
    from bass_rust import VectorClock

    if getattr(TileContext, "_drain_patch_installed", False):
        return

    def patched_drain_and_barrier(self, tick_clock, wait_clock):
        nc = self.nc
        gc = tick_clock.global_clock
        for i in range(27):
            v = gc[i]
            if v > 0:
                single = [0] * 27
                single[i] = v
                d = nc.sync.drain()
                wait_clock.add_sem_waits(
                    d.ins, ScopedClock({None: VectorClock(single)})
                )
        nc.all_engine_barrier()
        assert self.sems is not None
        popped = nc._tile_sem_poison_stack.pop()
        assert popped is self._sem_poison
        nc.clear_and_free_semaphores(list(self.sems.allocated().values()))
        nc.all_engine_barrier()

    TileContext._drain_and_barrier = patched_drain_and_barrier
    TileContext._drain_patch_installed = True


def _split_excess_waits(nc, max_waits=1):
    """This walrus build rejects instructions carrying more than ~2 sync
    waits. Move excess waits onto no-op instructions inserted immediately
    before the offender on the same engine (same semantics: the engine
    performs the same waits, in order, before executing the instruction)."""
    import concourse.mybir as mybir

    n_split = 0
    for fn in nc.m.functions:
        for bb in fn.blocks:
            insts = list(bb.instructions)
            new = []
            changed = False
            for inst in insts:
                si = inst.sync_info
                waits = list(si.on_wait) if si is not None else []
                if len(waits) > max_waits:
                    changed = True
                    n_split += 1
                    excess = waits[:-max_waits]
                    keep = waits[-max_waits:]
                    for i in range(0, len(excess), max_waits):
                        chunk = excess[i : i + max_waits]
                        nop = mybir.InstNoOp(
                            name=nc.get_next_instruction_name(),
                            sync_info=mybir.SyncInfo(
                                on_wait=chunk, on_update=[]
                            ),
                            bass_nofuse=True,
                            engine=inst.engine,
                        )
                        new.append(nop)
                    inst.sync_info = mybir.SyncInfo(
                        on_wait=keep, on_update=list(si.on_update)
                    )
                new.append(inst)
            if changed:
                bb.instructions = new
    return n_split


def build_nc():
    import concourse.bass as bass
    import concourse.mybir as mybir
    from concourse.bass import ts
    from concourse.masks import make_identity
    from concourse.tile import TileContext

    _install_tile_drain_patch()

    DT = mybir.dt.float32
    # fp16 keeps the same 11 explicit mantissa bits as fp32r (~2.4e-4 rel
    # err) but streams at full PE rate with fast weight loads; fp32r needs
    # every operand produced by a float32r-typed instruction (the rounding).
    MDT = {
        "f16": mybir.dt.float16,
        "f32r": mybir.dt.float32r,
        "f32": DT,
    }[PRECISION]
    nc = bass.Bass("TRN2", num_devices=NCORES)

    BF = mybir.dt.bfloat16
    # W ships as bf16 (sign-exact truncation of fp32 -- only its sign is
    # used) and is transposed by the DMA xbar on load, killing both the PE
    # transpose traffic and half the HBM stream. xT/wsT/f are pre-rounded
    # to the fp32r grid on the host and declared float32r so the DMA is a
    # valid fp32r producer for the matmuls.
    wbf_ext = nc.dram_tensor("wbf", [OS, IN], BF, kind="ExternalInput").ap()
    xT_ext = nc.dram_tensor("xT", [IN, T], MDT, kind="ExternalInput").ap()
    wsT_ext = nc.dram_tensor("wsT", [R, OS], MDT, kind="ExternalInput").ap()
    f_ext = nc.dram_tensor("f", [R, IN], MDT, kind="ExternalInput").ap()
    out_ext = nc.dram_tensor("out", [T, OS], DT, kind="ExternalOutput").ap()

    with TileContext(nc) as tc:
        with (
            tc.tile_pool(name="const", bufs=1) as cpool,
            tc.tile_pool(name="wtpool", bufs=4) as wtpool,
            tc.tile_pool(name="spool", bufs=4) as spool,
            tc.tile_pool(name="wsgpool", bufs=5) as wsgpool,
            tc.tile_pool(name="outsb", bufs=2) as outsb,
            tc.tile_pool(name="vpsum", bufs=2, space="PSUM") as vpool,
            tc.tile_pool(name="opsum", bufs=2, space="PSUM") as opool,
        ):
            # tiny f/wsT preloads go FIRST on the sync ring (ahead of the
            # W transposes) so the value matmuls unblock immediately; xT
            # rides the gpsimd SWDGE queue so the ACT ring is free to start
            # the Sign LUT work at t=0.
            f_sb = cpool.tile([R, IN], MDT)
            nc.sync.dma_start(f_sb[:, :], f_ext[:, :])
            wsT_sb = cpool.tile([R, OS], MDT)
            nc.sync.dma_start(wsT_sb[:, :], wsT_ext[:, :])

            # resident xT: [128, 32, 256], block ib holds xT[ib*128:(ib+1)*128, :]
            xT_sb = cpool.tile([P, IN // P, T], MDT)
            xT_view = xT_ext.rearrange("(a p) t -> p a t", p=P)
            for c0 in range(0, IN // P, 8):
                nc.gpsimd.dma_start(
                    xT_sb[:, c0 : c0 + 8], xT_view[:, c0 : c0 + 8]
                )

            n_iblk = IN // P  # 32
            # out[t, o] accumulates in PSUM across all 32 i-blocks:
            # 2 token-blocks x [128, 1376] fp32 = 2x3 banks, + 2 value
            # strips = 8 PSUM banks exactly.
            out_ps = [
                opool.tile([P, OS], DT, tag="out_ps", name=f"out_ps{tb}")
                for tb in range(2)
            ]
            for ib in range(n_iblk):
                first = ib == 0
                last = ib == n_iblk - 1
                # W strip [i_blk=128, all 1376 out-features], transposed by
                # the DMA xbar straight out of DRAM.
                wT_bf = wtpool.tile([P, OS], BF, tag="wT_bf", name="wT_bf")
                nc.sync.dma_start_transpose(
                    wT_bf[:, :], wbf_ext[:, ts(ib, P)]
                )
                # first produce all three signed-weight strips, then fire
                # the six main matmuls as one dense PE burst
                wsgs = []
                for (o0, No) in O_CHUNKS:
                    value_ps = vpool.tile(
                        [P, No], DT, tag="value_ps", name="value_ps"
                    )
                    nc.tensor.matmul(
                        value_ps,
                        f_sb[:, ts(ib, P)],
                        wsT_sb[:, o0 : o0 + No],
                        start=True,
                        stop=True,
                    )
                    # s = sign(w) in {-1, 0, +1} via ACT's Sign LUT
                    # (sign(+-0) = 0, matching jnp.sign), then one DVE
                    # multiply: w_signed = s * value; the DVE output cast
                    # doubles as the precision rounding.
                    s_sb = spool.tile([P, No], DT, tag="s_sb", name="s_sb")
                    nc.scalar.activation(
                        s_sb,
                        wT_bf[:, o0 : o0 + No],
                        mybir.ActivationFunctionType.Sign,
                    )
                    wsg_sb = wsgpool.tile(
                        [P, No], MDT, tag="wsg_sb", name="wsg_sb"
                    )
                    nc.vector.tensor_mul(wsg_sb, s_sb, value_ps)
                    wsgs.append(wsg_sb)
                for tb in range(2):
                    for (o0, No), wsg_sb in zip(O_CHUNKS, wsgs):
                        nc.tensor.matmul(
                            out_ps[tb][:, o0 : o0 + No],
                            xT_sb[:, ib, ts(tb, P)],
                            wsg_sb,
                            start=first,
                            stop=last,
                        )
            for tb in range(2):
                o_sb = outsb.tile([P, OS], DT, tag="o_sb", name="o_sb")
                nc.scalar.copy(o_sb, out_ps[tb])
                nc.scalar.dma_start(out_ext[ts(tb, P), :], o_sb)

    _split_excess_waits(nc)
    return nc


_NC_CACHE = None


def round_f32r(a):
    """Cast a matmul operand to the active precision grid: np.float16 for
    f16 mode; fp32 bits rounded to 11 explicit mantissa bits (RNE) for
    f32r mode -- what the on-device fp32r cast would produce."""
    if PRECISION == "f16":
        return np.ascontiguousarray(a, dtype=np.float32).astype(np.float16)
    if PRECISION != "f32r":
        return a
    bits = np.ascontiguousarray(a, dtype=np.float32).view(np.uint32)
    drop = 12
    q = np.uint32(1 << drop)
    lsb = (bits >> drop) & 1
    rounded = (bits + (q >> 1) - 1 + lsb) & ~(q - np.uint32(1))
    return rounded.view(np.float32)


def make_in_maps(x, weight, weight_scale, input_factor):
    import ml_dtypes

    xT = round_f32r(
        np.ascontiguousarray(x.reshape(T, IN).T.astype(np.float32))
    )
    f = round_f32r(np.ascontiguousarray(input_factor.astype(np.float32)))
    # only sign(weight) is used downstream; the bf16 cast preserves it
    # exactly (including +-0 -> sign 0)
    wbf = np.ascontiguousarray(weight.astype(ml_dtypes.bfloat16))
    in_maps = []
    for c in range(NCORES):
        sl = slice(c * OS, (c + 1) * OS)
        in_maps.append(
            {
                "wbf": wbf[sl],
                "xT": xT,
                "wsT": round_f32r(
                    np.ascontiguousarray(
                        weight_scale[sl].T.astype(np.float32)
                    )
                ),
                "f": f,
            }
        )
    return in_maps


def gather_out(results):
    outs = [results[c]["out"] for c in range(NCORES)]
    full = np.concatenate(outs, axis=1)  # [T, OUT]
    return np.ascontiguousarray(full.reshape(B, S, OUT).astype(np.float32))


def kernel(x, weight, weight_scale, input_factor):
    global _NC_CACHE
    from concourse.bass_utils import run_bass_kernel_spmd

    if _NC_CACHE is None:
        _NC_CACHE = build_nc()
    nc = _NC_CACHE

    in_maps = make_in_maps(x, weight, weight_scale, input_factor)
    res = run_bass_kernel_spmd(nc, in_maps, core_ids=list(range(NCORES)))
    return gather_out(res.results)


if __name__ == "__main__":
    # quick self-run with random data
    rng = np.random.default_rng(0)
    x = rng.standard_normal((B, S, IN), dtype=np.float32)
    w = rng.standard_normal((OUT, IN), dtype=np.float32)
    ws = rng.standard_normal((OUT, R), dtype=np.float32)
    f = rng.standard_normal((R, IN), dtype=np.float32)
    out = kernel(x=x, weight=w, weight_scale=ws, input_factor=f)
    wv = ws @ f
    expected = np.einsum("bsi,oi->bso", x, np.sign(w) * wv)
    rel = np.abs(out - expected).max() / np.abs(expected).max()
    print("rel err:", rel)



# revision 2
# speedup vs baseline: 1.5378x; 1.5378x over previous
"""BitLinear kernel for Trainium2, 8-core column-parallel. v2.

Computes out = x @ (sign(W) * (weight_scale @ input_factor)).T
  x: [32, 8, 4096] f32, W: [11008, 4096] f32,
  weight_scale: [11008, 4] f32, input_factor: [4, 4096] f32
  -> out: [32, 8, 11008] f32

Sharding: column-parallel over out_features (11008 = 8 x 1376). Each core
gets its W / weight_scale row-shard plus replicated x / input_factor, and
produces out[:, core_slice]; host concatenates. No collectives.

v2 dataflow (vs v1: every MM ran cold+isolated at (219+N)/1.2 ns because
the ACT-sign -> DVE-mul chain starved the PE; HAM never warmed):
  - sign(W) is precomputed on HOST and shipped as fp16 (+-1/0 exact),
    pre-arranged chunk-major -- no DMA transpose, no ACT Sign pass.
  - o-chunk OUTER loop (512/512/352), i-blocks in groups of 4.
  - value strips v[i,o] = f.T @ wsT are K=4 matmuls; 4 i-blocks run
    CONCURRENTLY via tile_position row-tiling (rows 32j..32j+3), one
    PSUM bank each.
  - value PSUM -> SBUF fp16 copies split 2:2 between ACT and DVE, then
    ONE batched DVE tensor_mul (fp16 SBUF operands -> 2x mode) makes
    wsg = s * v for all 4 blocks.
  - 8 main MMs per group (4 i-blocks x 2 token-blocks) accumulate
    out[t, o-chunk] in PSUM across all 32 i-blocks.
  - PSUM: 4 value banks + 2x2 double-buffered out banks = 8 exactly.
  - value MM of group g+1 is issued BEFORE main MMs of group g so the
    copies of g+1 overlap main(g) and the PE never idles long enough
    for HAM to re-throttle.
"""

import sys

if "/opt/trn_rl_repo" not in sys.path:
    sys.path.insert(0, "/opt/trn_rl_repo")

import numpy as np

# ---------------------------------------------------------------------------
# problem constants (hardcoded per the self-contained-kernel contract)
B, S, IN, OUT, R = 32, 8, 4096, 11008, 4
T = B * S               # 256 tokens
NCORES = 8
OS = OUT // NCORES      # 1376 out-features per core
P = 128
N_IBLK = IN // P        # 32 i-blocks
NGRP = N_IBLK // 4      # 8 groups of 4 i-blocks
O_CHUNKS = [(0, 512), (512, 512), (1024, 352)]
# free-dim offsets of each chunk's region in the chunk-major s layout
S_OFF = [0, N_IBLK * 512, N_IBLK * 1024]
S_TOT = N_IBLK * OS     # 44032


def _install_tile_drain_patch():
    """This walrus build rejects >2 sync waits on one TPB_CTRL instruction;
    split the TileContext end-of-kernel drain into one drain per proc."""
    from concourse.tile import TileContext
    from concourse.vector_clock import ScopedClock
    from bass_rust import VectorClock

    if getattr(TileContext, "_drain_patch_installed", False):
        return

    def patched_drain_and_barrier(self, tick_clock, wait_clock):
        nc = self.nc
        gc = tick_clock.global_clock
        for i in range(27):
            v = gc[i]
            if v > 0:
                single = [0] * 27
                single[i] = v
                d = nc.sync.drain()
                wait_clock.add_sem_waits(
                    d.ins, ScopedClock({None: VectorClock(single)})
                )
        nc.all_engine_barrier()
        assert self.sems is not None
        popped = nc._tile_sem_poison_stack.pop()
        assert popped is self._sem_poison
        nc.clear_and_free_semaphores(list(self.sems.allocated().values()))
        nc.all_engine_barrier()

    TileContext._drain_and_barrier = patched_drain_and_barrier
    TileContext._drain_patch_installed = True


def _split_excess_waits(nc, max_waits=1):
    """This walrus build rejects instructions carrying more than ~2 sync
    waits. Move excess waits onto no-op instructions inserted immediately
    before the offender on the same engine (same semantics: the engine
    performs the same waits, in order, before executing the instruction)."""
    import concourse.mybir as mybir

    n_split = 0
    for fn in nc.m.functions:
        for bb in fn.blocks:
            insts = list(bb.instructions)
            new = []
            changed = False
            for inst in insts:
                si = inst.sync_info
                waits = list(si.on_wait) if si is not None else []
                if len(waits) > max_waits:
                    changed = True
                    n_split += 1
                    excess = waits[:-max_waits]
                    keep = waits[-max_waits:]
                    for i in range(0, len(excess), max_waits):
                        chunk = excess[i : i + max_waits]
                        nop = mybir.InstNoOp(
                            name=nc.get_next_instruction_name(),
                            sync_info=mybir.SyncInfo(
                                on_wait=chunk, on_update=[]
                            ),
                            bass_nofuse=True,
                            engine=inst.engine,
                        )
                        new.append(nop)
                    inst.sync_info = mybir.SyncInfo(
                        on_wait=keep, on_update=list(si.on_update)
                    )
                new.append(inst)
            if changed:
                bb.instructions = new
    return n_split


def build_nc():
    import concourse.bass as bass
    import concourse.mybir as mybir
    from concourse.bass import ts
    from concourse.tile import TileContext

    _install_tile_drain_patch()

    F32 = mybir.dt.float32
    F16 = mybir.dt.float16
    nc = bass.Bass("TRN2", num_devices=NCORES)

    # host-prearranged inputs, all [128, free] so DMAs are 1:1 plain loads
    s_ext = nc.dram_tensor("s", [P, S_TOT], F16, kind="ExternalInput").ap()
    xT_ext = nc.dram_tensor(
        "xT", [P, N_IBLK * T], F16, kind="ExternalInput"
    ).ap()
    frep_ext = nc.dram_tensor(
        "frep", [P, NGRP * P], F16, kind="ExternalInput"
    ).ap()
    wsrep_ext = nc.dram_tensor(
        "wsrep", [P, OS], F16, kind="ExternalInput"
    ).ap()
    out_ext = nc.dram_tensor("out", [T, OS], F32, kind="ExternalOutput").ap()

    with TileContext(nc) as tc:
        with (
            tc.tile_pool(name="const", bufs=1) as cpool,
            tc.tile_pool(name="vsb", bufs=2) as vsbpool,
            tc.tile_pool(name="wsgp", bufs=2) as wsgpool,
            tc.tile_pool(name="outsb", bufs=2) as outsb,
            tc.tile_pool(name="vpsum", bufs=1, space="PSUM") as vpool,
            tc.tile_pool(name="opsum", bufs=2, space="PSUM") as opool,
        ):
            # resident SBUF inputs
            frep_sb = cpool.tile([P, NGRP, P], F16)
            wsrep_sb = cpool.tile([P, OS], F16)
            s_sb = cpool.tile([P, S_TOT], F16)
            xT_sb = cpool.tile([P, N_IBLK, T], F16)

            # prefetch order: tiny stationaries first, then chunk-0 s and
            # xT interleaved per-group on two HWDGE rings so the first
            # main MMs unblock ~4us in; later chunks stream behind.
            nc.sync.dma_start(frep_sb[:, :, :], frep_ext.rearrange("p (g i) -> p g i", g=NGRP))
            nc.sync.dma_start(wsrep_sb[:, :], wsrep_ext[:, :])
            xT_view = xT_ext.rearrange("p (a t) -> p a t", a=N_IBLK)
            for g in range(NGRP):
                c0, Nc = O_CHUNKS[0]
                lo = S_OFF[0] + g * 4 * Nc
                nc.sync.dma_start(
                    s_sb[:, lo : lo + 4 * Nc], s_ext[:, lo : lo + 4 * Nc]
                )
                nc.scalar.dma_start(
                    xT_sb[:, g * 4 : (g + 1) * 4], xT_view[:, g * 4 : (g + 1) * 4]
                )
            for c in (1, 2):
                c0, Nc = O_CHUNKS[c]
                for g in range(NGRP):
                    lo = S_OFF[c] + g * 4 * Nc
                    q = nc.sync if g % 2 == 0 else nc.scalar
                    q.dma_start(
                        s_sb[:, lo : lo + 4 * Nc], s_ext[:, lo : lo + 4 * Nc]
                    )

            for c, (c0, Nc) in enumerate(O_CHUNKS):
                out_ps = [
                    opool.tile([P, Nc], F32, tag=f"out{tb}", name=f"out_ps{tb}")
                    for tb in range(2)
                ]
                vps_next = None
                for g in range(NGRP):
                    if g == 0:
                        # value strips for group 0 of this chunk: 4 K=4
                        # matmuls run concurrently in distinct 32-row
                        # groups of the PE array
                        vps_next = [
                            vpool.tile([P, Nc], F32, tag=f"v{j}", name=f"v{j}")
                            for j in range(4)
                        ]
                        for j in range(4):
                            nc.tensor.matmul(
                                vps_next[j],
                                frep_sb[32 * j : 32 * j + 4, g, :],
                                wsrep_sb[32 * j : 32 * j + 4, c0 : c0 + Nc],
                                start=True,
                                stop=True,
                                tile_position=(32 * j, 0),
                            )
                    vps = vps_next

                    # value PSUM -> SBUF fp16, split 2:2 ACT/DVE
                    v_sb = vsbpool.tile([P, 4 * Nc], F16, tag="v_sb", name="v_sb")
                    nc.scalar.copy(v_sb[:, 0 * Nc : 1 * Nc], vps[0])
                    nc.scalar.copy(v_sb[:, 1 * Nc : 2 * Nc], vps[1])
                    nc.vector.tensor_copy(v_sb[:, 2 * Nc : 3 * Nc], vps[2])
                    nc.vector.tensor_copy(v_sb[:, 3 * Nc : 4 * Nc], vps[3])

                    # batched signed-weight build: one DVE tensor_mul in 2x
                    # mode over all 4 blocks (both operands fp16 SBUF)
                    wsg = wsgpool.tile([P, 4 * Nc], F16, tag="wsg", name="wsg")
                    lo = S_OFF[c] + g * 4 * Nc
                    nc.vector.tensor_mul(wsg, s_sb[:, lo : lo + 4 * Nc], v_sb)

                    # issue NEXT group's value MMs before this group's main
                    # burst so their PSUM banks refill while main runs
                    if g + 1 < NGRP:
                        vps_next = [
                            vpool.tile([P, Nc], F32, tag=f"v{j}", name=f"v{j}")
                            for j in range(4)
                        ]
                        for j in range(4):
                            nc.tensor.matmul(
                                vps_next[j],
                                frep_sb[32 * j : 32 * j + 4, g + 1, :],
                                wsrep_sb[32 * j : 32 * j + 4, c0 : c0 + Nc],
                                start=True,
                                stop=True,
                                tile_position=(32 * j, 0),
                            )

                    # main burst: 8 dense MMs accumulating out[t, o-chunk]
                    for j in range(4):
                        ib = g * 4 + j
                        for tb in range(2):
                            nc.tensor.matmul(
                                out_ps[tb],
                                xT_sb[:, ib, ts(tb, P)],
                                wsg[:, j * Nc : (j + 1) * Nc],
                                start=(g == 0 and j == 0),
                                stop=(g == NGRP - 1 and j == 3),
                            )

                # evacuate the finished chunk (next chunk uses the other
                # out PSUM buffers, so this overlaps)
                for tb in range(2):
                    o_sb = outsb.tile(
                        [P, Nc], F32, tag=f"osb{tb}", name=f"o_sb{tb}"
                    )
                    nc.scalar.copy(o_sb, out_ps[tb])
                    nc.sync.dma_start(out_ext[ts(tb, P), c0 : c0 + Nc], o_sb)

    _split_excess_waits(nc)
    return nc


_NC_CACHE = None


def make_in_maps(x, weight, weight_scale, input_factor):
    xf = np.ascontiguousarray(x.reshape(T, IN)).astype(np.float32)
    # xT_arr[p, ib*T + t] = x[t, ib*128 + p]
    xT_arr = (
        xf.T.reshape(N_IBLK, P, T).transpose(1, 0, 2).reshape(P, N_IBLK * T)
    ).astype(np.float16)

    f32 = input_factor.astype(np.float32)
    in_maps = []
    for core in range(NCORES):
        sl = slice(core * OS, (core + 1) * OS)
        w_c = np.asarray(weight[sl], dtype=np.float32)      # [OS, IN]
        ws_c = np.asarray(weight_scale[sl], dtype=np.float32)  # [OS, R]

        # s chunk-major: s_arr[p, S_OFF[c] + ib*4? -> (ib within group):
        # region c holds [ib, Nc] blocks: s_arr[p, S_OFF[c] + ib*Nc + u]
        #   = sign(W[c0+u? no: = sign(w_c[o, i]) at i = ib*128+p, o = c0+u
        sT = np.sign(w_c).T.astype(np.float16)              # [IN, OS]
        sT3 = sT.reshape(N_IBLK, P, OS)                     # [ib, p, o]
        parts = []
        for (c0, Nc) in O_CHUNKS:
            # [ib, p, Nc] -> [p, ib, Nc]
            parts.append(
                sT3[:, :, c0 : c0 + Nc].transpose(1, 0, 2).reshape(P, -1)
            )
        s_arr = np.ascontiguousarray(np.concatenate(parts, axis=1))

        # frep[32j + r, g*128 + ii] = f[r, (4g+j)*128 + ii]
        frep = np.zeros((P, NGRP * P), dtype=np.float16)
        for j in range(4):
            for r in range(R):
                for g in range(NGRP):
                    frep[32 * j + r, g * P : (g + 1) * P] = f32[
                        r, (4 * g + j) * P : (4 * g + j + 1) * P
                    ]
        # wsrep[32j + r, o] = ws_c[o, r]
        wsrep = np.zeros((P, OS), dtype=np.float16)
        for j in range(4):
            for r in range(R):
                wsrep[32 * j + r, :] = ws_c[:, r]

        in_maps.append(
            {"s": s_arr, "xT": xT_arr, "frep": frep, "wsrep": wsrep}
        )
    return in_maps


def gather_out(results):
    outs = [results[c]["out"] for c in range(NCORES)]
    full = np.concatenate(outs, axis=1)  # [T, OUT]
    return np.ascontiguousarray(full.reshape(B, S, OUT).astype(np.float32))


def kernel(x, weight, weight_scale, input_factor):
    global _NC_CACHE
    from concourse.bass_utils import run_bass_kernel_spmd

    if _NC_CACHE is None:
        _NC_CACHE = build_nc()
    nc = _NC_CACHE

    in_maps = make_in_maps(x, weight, weight_scale, input_factor)
    res = run_bass_kernel_spmd(nc, in_maps, core_ids=list(range(NCORES)))
    return gather_out(res.results)


if __name__ == "__main__":
    # quick self-run with random data
    rng = np.random.default_rng(0)
    x = rng.standard_normal((B, S, IN), dtype=np.float32)
    w = rng.standard_normal((OUT, IN), dtype=np.float32)
    ws = rng.standard_normal((OUT, R), dtype=np.float32)
    f = rng.standard_normal((R, IN), dtype=np.float32)
    out = kernel(x=x, weight=w, weight_scale=ws, input_factor=f)
    wv = ws @ f
    expected = np.einsum("bsi,oi->bso", x, np.sign(w) * wv)
    rel = np.abs(out - expected).max() / np.abs(expected).max()
    print("rel err:", rel)
